# revision 36
# baseline (speedup 1.0000x reference)
"""Embedding-lookup-sum kernel for TRN2 (8 NeuronCores, data-parallel).

out[n] = sum_f emb_tables[f, indices[n, f]]   (N=65536 tokens, F=8, D=256)

Strategy:
  - Shard tokens across 8 cores (8192 tokens/core); replicate the tables.
  - Fuse the 8 per-feature tables into one [8*1026, 256] DRAM table with the
    feature offset folded into the index on the host (idx + 1026*f).
  - Quantize the table to int8 with one global scale (absmax/126). The 8-way
    sum of int8 rows is exact in f16 (|sum| <= 1016 < 2048), so the host
    multiplies the f16 output by 1/scale afterwards. 256B gather rows halve
    both HBM traffic and SDMA per-descriptor time vs f16.
  - Per core: 8 tiles x 1024 tokens; per tile one 1024-idx dma_gather per
    (queue, feature) pair — queue q owns features q and q+4 (4 SWDGE queues).
    DVE folds the 8 feature blocks with a 4-add tree (int8 pair adds -> f16),
    sync DMAs the f16 sums out in 4KB/partition lines (token p*8+c at
    partition p, chunk c via a host-side index permutation).
  - A warmup gather absorbs the Q7 ucode cold start while the (split) idx
    load is in flight.

Measured bottleneck: dma_gather descriptor generation is effectively SERIAL
on the Q7s at ~2.8us per 1024-idx call (~2.7ns/desc: scalar idx unpack +
vector desc pushes), so 64 calls ~ 179us dominates; DMA drain (18.4ns/desc
per engine) and the DVE add tree (12.4us/tile) hide underneath. 2048-idx
calls exceed the per-queue descriptor ring and hang the device; bigger
dynamic_dma_scratch_size does not raise the effective ring.
"""

import sys

sys.path.insert(0, "/opt/trn_rl_repo")

import numpy as np

N_TOKENS = 65536
F = 8
VOCAB = 1026
D = 256
NCORES = 8
TPC = N_TOKENS // NCORES  # tokens per core = 8192
# Tile sizes: big tiles for low overhead, small trailing tiles so the final
# (gather -> DVE chain -> out DMA) dependency tail is short.
TSIZES = [1024] * 7 + [256] * 4
assert sum(TSIZES) == TPC and all(sz % 128 == 0 for sz in TSIZES)
TOFFS = [sum(TSIZES[:j]) for j in range(len(TSIZES))]
NTILES = len(TSIZES)
NQUEUES = 4
GPQ = 2  # gather calls per queue per tile (features q and q+4)
# idx column offset of (tile j, queue q, call k): 16-wrapped positions
TCOLS = [2 * sz // 16 for sz in TSIZES]  # idx cols per (tile, queue)
TCOL_OFF = [sum(NQUEUES * c for c in TCOLS[:j]) for j in range(NTILES)]
IDX_COLS = sum(NQUEUES * c for c in TCOLS)  # total idx columns
NBUFG = 3  # gather buffer depth (tiles in flight)
SCRATCH = 32768  # dynamic DMA descriptor carveout bytes
WARMUP = True

GATHER_DT = "i8"  # "i8" (int8 + host scale) or "f16"
Q_SCALE_NUM = 126.0  # int8 quantization: scale = Q_SCALE_NUM / absmax


def build_nc(compile_: bool = True):
    import concourse.bacc as bacc
    import concourse.mybir as mybir
    from concourse.library_config import mlp
    from contextlib import ExitStack

    i8 = GATHER_DT == "i8"
    gdt = mybir.dt.int8 if i8 else mybir.dt.float16
    nbufg = NBUFG if i8 else 2

    nc = bacc.Bacc(
        "TRN2",
        debug=False,
        num_swdge_queues=NQUEUES,
        dynamic_dma_scratch_size=SCRATCH,
    )
    CHMAX = max(TSIZES) // 128

    tables = nc.dram_tensor("tables", [F * VOCAB, D], gdt, kind="ExternalInput")
    idx = nc.dram_tensor("idx", [128, IDX_COLS], mybir.dt.int16, kind="ExternalInput")
    out = nc.dram_tensor("out", [TPC, D], mybir.dt.float16, kind="ExternalOutput")

    with ExitStack() as ctx:
        idx_sb = ctx.enter_context(
            nc.sbuf_tensor("idx_sb", [128, IDX_COLS], mybir.dt.int16)
        )
        widx = ctx.enter_context(nc.sbuf_tensor("widx", [128, 8], mybir.dt.int16))
        gw = ctx.enter_context(nc.sbuf_tensor("gw", [128, 1, D], gdt))
        # g[b]: 8 feature blocks of ch chunks each (ch = tile_size/128);
        # queue q's two gathers fill blocks for features q and q+4.
        g = [
            ctx.enter_context(nc.sbuf_tensor(f"g{b}", [128, F * CHMAX, D], gdt))
            for b in range(nbufg)
        ]
        # s[b2]: f16 partial sums; final tile result lands in s[b2][:, 0:ch, :]
        s = [
            ctx.enter_context(
                nc.sbuf_tensor(f"s{b}", [128, 4 * CHMAX, D], mybir.dt.float16)
            )
            for b in range(2)
        ]
        s_w = ctx.enter_context(nc.semaphore("s_w"))
        s_wg = ctx.enter_context(nc.semaphore("s_wg"))
        s_idx0 = ctx.enter_context(nc.semaphore("s_idx0"))
        s_idxr = ctx.enter_context(nc.semaphore("s_idxr"))
        # Per-(buffer, queue) gather sems: a sem may only be updated from one
        # SWDGE queue, and count-based waits need all DMAs on a sem to be
        # "all issued so far" (completions can reorder).
        s_g = [
            [ctx.enter_context(nc.semaphore(f"s_g{b}_{q}")) for q in range(NQUEUES)]
            for b in range(nbufg)
        ]
        s_ch = ctx.enter_context(nc.semaphore("s_ch"))  # +3/tile (c1, c2, c3)
        s_sum = ctx.enter_context(nc.semaphore("s_sum"))  # +1/tile (c4)
        s_out = [ctx.enter_context(nc.semaphore(f"s_out{b}")) for b in range(2)]

        with nc.Block(no_gpsimd_drain=True) as block:

            @block.gpsimd
            def _(gp):
                gp.load_library(mlp)
                regs = {sz: gp.to_reg(sz) for sz in sorted(set(TSIZES))}
                if WARMUP:
                    wreg = gp.to_reg(16)
                    # Warmup: page in the dma_gather ucode while the idx DMA
                    # is in flight. Reads table row 0 (widx zeroed by DVE).
                    # 16 idxs: just enough to touch the gen code path.
                    gp.wait_ge(s_w, 1)
                    gp.dma_gather(
                        gw[:], tables[:], widx[:, 0:1], 16, wreg, D,
                        queue_num=0,
                    ).then_inc(s_wg, 16)
                for t in range(NTILES):
                    b = t % nbufg
                    sz = TSIZES[t]
                    ch = sz // 128
                    gcols = sz // 16
                    gp.wait_ge(s_idx0 if t == 0 else s_idxr, 16)
                    if t >= nbufg:
                        # g[b] free once tile t-nbufg's c3 consumed its last block
                        gp.wait_ge(s_ch, 3 * (t - nbufg + 1))
                    for q in range(NQUEUES):
                        for k in range(GPQ):
                            c0 = TCOL_OFF[t] + q * TCOLS[t] + k * gcols
                            ch0 = (q * 2 + k) * ch
                            gp.dma_gather(
                                g[b][:, ch0 : ch0 + ch, :],
                                tables[:],
                                idx_sb[:, c0 : c0 + gcols],
                                sz,
                                regs[sz],
                                D,
                                queue_num=q,
                                single_packet=False,
                            ).then_inc(s_g[b][q], 16)

            @block.vector
            def _(v):
                if WARMUP:
                    v.memzero(widx[:]).then_inc(s_w, 1)
                for t in range(NTILES):
                    b = t % nbufg
                    b2 = t % 2
                    ch = TSIZES[t] // 128
                    # s[b2] free once tile t-2's out DMA read it
                    if t >= 2:
                        v.wait_ge(s_out[b2], 16 * (t // 2))
                        # ...and tile t-2's c4 (same buffer) fully retired
                        v.wait_ge(s_sum, t - 1)
                    # Serial accumulation: each queue's block folds in as soon
                    # as its gathers land, so only c3+c4 trail the last gather.
                    v.wait_ge(s_g[b][0], 16 * GPQ * (t // nbufg + 1))
                    v.wait_ge(s_g[b][1], 16 * GPQ * (t // nbufg + 1))
                    # c1: (f0+f1 | f4+f5) -> s[0:2ch]
                    v.tensor_add(
                        s[b2][:, 0 : 2 * ch, :],
                        g[b][:, 0 : 2 * ch, :],
                        g[b][:, 2 * ch : 4 * ch, :],
                    ).then_inc(s_ch, 1)
                    # c2: += (f2 | f6)
                    v.wait_ge(s_g[b][2], 16 * GPQ * (t // nbufg + 1))
                    v.wait_ge(s_ch, 3 * t + 1)
                    v.tensor_add(
                        s[b2][:, 0 : 2 * ch, :],
                        s[b2][:, 0 : 2 * ch, :],
                        g[b][:, 4 * ch : 6 * ch, :],
                    ).then_inc(s_ch, 1)
                    # c3: += (f3 | f7)
                    v.wait_ge(s_g[b][3], 16 * GPQ * (t // nbufg + 1))
                    v.wait_ge(s_ch, 3 * t + 2)
                    v.tensor_add(
                        s[b2][:, 0 : 2 * ch, :],
                        s[b2][:, 0 : 2 * ch, :],
                        g[b][:, 6 * ch : 8 * ch, :],
                    ).then_inc(s_ch, 1)
                    # c4: fold halves -> s[0:ch]
                    v.wait_ge(s_ch, 3 * t + 3)
                    v.tensor_add(
                        s[b2][:, 0:ch, :],
                        s[b2][:, 0:ch, :],
                        s[b2][:, ch : 2 * ch, :],
                    ).then_inc(s_sum, 1)

            @block.sync
            def _(sy):
                c_t0 = NQUEUES * TCOLS[0]
                sy.dma_start(idx_sb[:, 0:c_t0], idx[:, 0:c_t0]).then_inc(s_idx0, 16)
                sy.dma_start(idx_sb[:, c_t0:], idx[:, c_t0:]).then_inc(s_idxr, 16)
                for t in range(NTILES):
                    b2 = t % 2
                    ch = TSIZES[t] // 128
                    sy.wait_ge(s_sum, t + 1)
                    # token p*ch+c sits at partition p, chunk c -> 4KB HBM rows
                    dst = out[TOFFS[t] : TOFFS[t] + TSIZES[t], :].rearrange(
                        "(p c) d -> p c d", p=128
                    )
                    sy.dma_start(dst, s[b2][:, 0:ch, :]).then_inc(s_out[b2], 16)
                if WARMUP:
                    sy.wait_ge(s_wg, 16)
                n_par = [sum(1 for t in range(NTILES) if t % 2 == b) for b in range(2)]
                for b in range(2):
                    sy.wait_ge(s_out[b], 16 * n_par[b])

    if compile_:
        nc.compile()
    return nc


def make_in_maps(indices: np.ndarray, emb_tables: np.ndarray):
    """Host-side sharding + index marshalling into dma_gather's layout.

    Returns (in_maps, inv_scale)."""
    idx = np.asarray(indices).astype(np.int64)  # [N_TOKENS, F]
    tab = np.ascontiguousarray(np.asarray(emb_tables), dtype=np.float32).reshape(
        F * VOCAB, D
    )
    if GATHER_DT == "i8":
        absmax = float(np.abs(tab).max())
        scale = Q_SCALE_NUM / absmax if absmax > 0 else 1.0
        qtab = np.clip(np.rint(tab * scale), -127, 127).astype(np.int8)
        inv_scale = np.float32(1.0 / scale)
    else:
        qtab = tab.astype(np.float16)
        inv_scale = np.float32(1.0)
    fused = (idx + (np.arange(F, dtype=np.int64) * VOCAB)[None, :]).astype(np.int16)

    in_maps = []
    for c in range(NCORES):
        sh = fused[c * TPC : (c + 1) * TPC]  # [TPC, F]
        cols = np.empty((16, IDX_COLS), dtype=np.int16)
        for t in range(NTILES):
            sz, ch = TSIZES[t], TSIZES[t] // 128
            # gather position i (in a feature block) holds token (i%128)*ch + i//128
            perm = (np.arange(sz) % 128) * ch + np.arange(sz) // 128
            tokp = sh[TOFFS[t] : TOFFS[t] + sz][perm]  # [sz, F]
            # queue q gathers features (q, q+4): block [q, 2*sz]
            blocks = np.concatenate(
                [tokp[:, 0:NQUEUES], tokp[:, NQUEUES:F]], axis=0
            ).T  # [q, 2*sz]
            # position j -> partition j%16, column j//16
            wrapped = blocks.reshape(NQUEUES, 2 * sz // 16, 16).transpose(2, 0, 1)
            c0 = TCOL_OFF[t]
            cols[:, c0 : c0 + NQUEUES * TCOLS[t]] = wrapped.reshape(16, -1)
        idx128 = np.ascontiguousarray(np.tile(cols, (8, 1)))
        in_maps.append({"tables": qtab, "idx": idx128})
    return in_maps, inv_scale


_NC = None


def kernel(indices: np.ndarray, emb_tables: np.ndarray) -> np.ndarray:
    global _NC
    from concourse.bass_utils import run_bass_kernel_spmd

    in_maps, inv_scale = make_in_maps(indices, emb_tables)
    if _NC is None:
        _NC = build_nc()
    res = run_bass_kernel_spmd(_NC, in_maps, core_ids=list(range(NCORES)))
    outs = [np.asarray(res.results[c]["out"]) for c in range(NCORES)]
    full = np.concatenate(outs, axis=0).astype(np.float32) * inv_scale
    return full.reshape(1, N_TOKENS, D)


# revision 37
# speedup vs baseline: 1.0163x; 1.0163x over previous
"""Embedding-lookup-sum kernel for TRN2 (8 NeuronCores, data-parallel).

out[n] = sum_f emb_tables[f, indices[n, f]]   (N=65536 tokens, F=8, D=256)

Strategy:
  - Shard tokens across 8 cores (8192 tokens/core); replicate the tables.
  - Fuse the 8 per-feature tables into one [8*1026, 256] DRAM table with the
    feature offset folded into the index on the host (idx + 1026*f).
  - Quantize the table to int8 with one global scale (absmax/126). The 8-way
    sum of int8 rows is exact in f16 (|sum| <= 1016 < 2048), so the host
    multiplies the f16 output by 1/scale afterwards. 256B gather rows halve
    both HBM traffic and SDMA per-descriptor time vs f16.
  - Per core: 8 tiles x 1024 tokens; per tile one 1024-idx dma_gather per
    (queue, feature) pair — queue q owns features q and q+4 (4 SWDGE queues).
    DVE folds the 8 feature blocks with a 4-add tree (int8 pair adds -> f16),
    sync DMAs the f16 sums out in 4KB/partition lines (token p*8+c at
    partition p, chunk c via a host-side index permutation).
  - A warmup gather absorbs the Q7 ucode cold start while the (split) idx
    load is in flight.

Measured bottleneck: dma_gather descriptor generation is effectively SERIAL
on the Q7s at ~2.8us per 1024-idx call (~2.7ns/desc: scalar idx unpack +
vector desc pushes), so 64 calls ~ 179us dominates; DMA drain (18.4ns/desc
per engine) and the DVE add tree (12.4us/tile) hide underneath. 2048-idx
calls exceed the per-queue descriptor ring and hang the device; bigger
dynamic_dma_scratch_size does not raise the effective ring.
"""

import sys

sys.path.insert(0, "/opt/trn_rl_repo")

import numpy as np

N_TOKENS = 65536
F = 8
VOCAB = 1026
D = 256
NCORES = 8
TPC = N_TOKENS // NCORES  # tokens per core = 8192
# Tile sizes. 1024-idx gathers are the largest the descriptor ring allows;
# uniform 1024-token tiles minimize call count (64), which matters because
# desc-gen is serial and each call carries ~0.35us fixed cost. (Tried
# [1024]*7+[256]*4 to shorten the dependency tail: the 24 extra calls cost
# more than the ~4us tail saving.)
TSIZES = [1024] * 8
assert sum(TSIZES) == TPC and all(sz % 128 == 0 for sz in TSIZES)
TOFFS = [sum(TSIZES[:j]) for j in range(len(TSIZES))]
NTILES = len(TSIZES)
NQUEUES = 4
GPQ = 2  # gather calls per queue per tile (features q and q+4)
# idx column offset of (tile j, queue q, call k): 16-wrapped positions
TCOLS = [2 * sz // 16 for sz in TSIZES]  # idx cols per (tile, queue)
TCOL_OFF = [sum(NQUEUES * c for c in TCOLS[:j]) for j in range(NTILES)]
IDX_COLS = sum(NQUEUES * c for c in TCOLS)  # total idx columns
NBUFG = 3  # gather buffer depth (tiles in flight)
SCRATCH = 32768  # dynamic DMA descriptor carveout bytes
WARMUP = True

GATHER_DT = "i8"  # "i8" (int8 + host scale) or "f16"
Q_SCALE_NUM = 126.0  # int8 quantization: scale = Q_SCALE_NUM / absmax


def build_nc(compile_: bool = True):
    import concourse.bacc as bacc
    import concourse.mybir as mybir
    from concourse.library_config import mlp
    from contextlib import ExitStack

    i8 = GATHER_DT == "i8"
    gdt = mybir.dt.int8 if i8 else mybir.dt.float16
    nbufg = NBUFG if i8 else 2

    nc = bacc.Bacc(
        "TRN2",
        debug=False,
        num_swdge_queues=NQUEUES,
        dynamic_dma_scratch_size=SCRATCH,
    )
    CHMAX = max(TSIZES) // 128

    tables = nc.dram_tensor("tables", [F * VOCAB, D], gdt, kind="ExternalInput")
    idx = nc.dram_tensor("idx", [128, IDX_COLS], mybir.dt.int16, kind="ExternalInput")
    out = nc.dram_tensor("out", [TPC, D], mybir.dt.float16, kind="ExternalOutput")

    with ExitStack() as ctx:
        idx_sb = ctx.enter_context(
            nc.sbuf_tensor("idx_sb", [128, IDX_COLS], mybir.dt.int16)
        )
        widx = ctx.enter_context(nc.sbuf_tensor("widx", [128, 8], mybir.dt.int16))
        gw = ctx.enter_context(nc.sbuf_tensor("gw", [128, 1, D], gdt))
        # g[b]: 8 feature blocks of ch chunks each (ch = tile_size/128);
        # queue q's two gathers fill blocks for features q and q+4.
        g = [
            ctx.enter_context(nc.sbuf_tensor(f"g{b}", [128, F * CHMAX, D], gdt))
            for b in range(nbufg)
        ]
        # s[b2]: f16 partial sums; final tile result lands in s[b2][:, 0:ch, :]
        s = [
            ctx.enter_context(
                nc.sbuf_tensor(f"s{b}", [128, 4 * CHMAX, D], mybir.dt.float16)
            )
            for b in range(2)
        ]
        s_w = ctx.enter_context(nc.semaphore("s_w"))
        s_wg = ctx.enter_context(nc.semaphore("s_wg"))
        s_idx0 = ctx.enter_context(nc.semaphore("s_idx0"))
        s_idxr = ctx.enter_context(nc.semaphore("s_idxr"))
        # Per-(buffer, queue) gather sems: a sem may only be updated from one
        # SWDGE queue, and count-based waits need all DMAs on a sem to be
        # "all issued so far" (completions can reorder).
        s_g = [
            [ctx.enter_context(nc.semaphore(f"s_g{b}_{q}")) for q in range(NQUEUES)]
            for b in range(nbufg)
        ]
        s_ch = ctx.enter_context(nc.semaphore("s_ch"))  # +3/tile (c1, c2, c3)
        s_sum = ctx.enter_context(nc.semaphore("s_sum"))  # +1/tile (c4)
        s_out = [ctx.enter_context(nc.semaphore(f"s_out{b}")) for b in range(2)]

        with nc.Block(no_gpsimd_drain=True) as block:

            @block.gpsimd
            def _(gp):
                gp.load_library(mlp)
                regs = {sz: gp.to_reg(sz) for sz in sorted(set(TSIZES))}
                if WARMUP:
                    wreg = gp.to_reg(16)
                    # Warmup: page in the dma_gather ucode while the idx DMA
                    # is in flight. Reads table row 0 (widx zeroed by DVE).
                    # 16 idxs: just enough to touch the gen code path.
                    gp.wait_ge(s_w, 1)
                    gp.dma_gather(
                        gw[:], tables[:], widx[:, 0:1], 16, wreg, D,
                        queue_num=0,
                    ).then_inc(s_wg, 16)
                for t in range(NTILES):
                    b = t % nbufg
                    sz = TSIZES[t]
                    ch = sz // 128
                    gcols = sz // 16
                    gp.wait_ge(s_idx0 if t == 0 else s_idxr, 16)
                    if t >= nbufg:
                        # g[b] free once tile t-nbufg's c3 consumed its last block
                        gp.wait_ge(s_ch, 3 * (t - nbufg + 1))
                    for q in range(NQUEUES):
                        for k in range(GPQ):
                            c0 = TCOL_OFF[t] + q * TCOLS[t] + k * gcols
                            ch0 = (q * 2 + k) * ch
                            gp.dma_gather(
                                g[b][:, ch0 : ch0 + ch, :],
                                tables[:],
                                idx_sb[:, c0 : c0 + gcols],
                                sz,
                                regs[sz],
                                D,
                                queue_num=q,
                                single_packet=False,
                            ).then_inc(s_g[b][q], 16)

            @block.vector
            def _(v):
                if WARMUP:
                    v.memzero(widx[:]).then_inc(s_w, 1)
                for t in range(NTILES):
                    b = t % nbufg
                    b2 = t % 2
                    ch = TSIZES[t] // 128
                    # s[b2] free once tile t-2's out DMA read it
                    if t >= 2:
                        v.wait_ge(s_out[b2], 16 * (t // 2))
                        # ...and tile t-2's c4 (same buffer) fully retired
                        v.wait_ge(s_sum, t - 1)
                    # Serial accumulation: each queue's block folds in as soon
                    # as its gathers land, so only c3+c4 trail the last gather.
                    v.wait_ge(s_g[b][0], 16 * GPQ * (t // nbufg + 1))
                    v.wait_ge(s_g[b][1], 16 * GPQ * (t // nbufg + 1))
                    # c1: (f0+f1 | f4+f5) -> s[0:2ch]
                    v.tensor_add(
                        s[b2][:, 0 : 2 * ch, :],
                        g[b][:, 0 : 2 * ch, :],
                        g[b][:, 2 * ch : 4 * ch, :],
                    ).then_inc(s_ch, 1)
                    # c2: += (f2 | f6)
                    v.wait_ge(s_g[b][2], 16 * GPQ * (t // nbufg + 1))
                    v.wait_ge(s_ch, 3 * t + 1)
                    v.tensor_add(
                        s[b2][:, 0 : 2 * ch, :],
                        s[b2][:, 0 : 2 * ch, :],
                        g[b][:, 4 * ch : 6 * ch, :],
                    ).then_inc(s_ch, 1)
                    # c3: += (f3 | f7)
                    v.wait_ge(s_g[b][3], 16 * GPQ * (t // nbufg + 1))
                    v.wait_ge(s_ch, 3 * t + 2)
                    v.tensor_add(
                        s[b2][:, 0 : 2 * ch, :],
                        s[b2][:, 0 : 2 * ch, :],
                        g[b][:, 6 * ch : 8 * ch, :],
                    ).then_inc(s_ch, 1)
                    # c4: fold halves -> s[0:ch]
                    v.wait_ge(s_ch, 3 * t + 3)
                    v.tensor_add(
                        s[b2][:, 0:ch, :],
                        s[b2][:, 0:ch, :],
                        s[b2][:, ch : 2 * ch, :],
                    ).then_inc(s_sum, 1)

            @block.sync
            def _(sy):
                c_t0 = NQUEUES * TCOLS[0]
                sy.dma_start(idx_sb[:, 0:c_t0], idx[:, 0:c_t0]).then_inc(s_idx0, 16)
                sy.dma_start(idx_sb[:, c_t0:], idx[:, c_t0:]).then_inc(s_idxr, 16)
                for t in range(NTILES):
                    b2 = t % 2
                    ch = TSIZES[t] // 128
                    sy.wait_ge(s_sum, t + 1)
                    # token p*ch+c sits at partition p, chunk c -> 4KB HBM rows
                    dst = out[TOFFS[t] : TOFFS[t] + TSIZES[t], :].rearrange(
                        "(p c) d -> p c d", p=128
                    )
                    sy.dma_start(dst, s[b2][:, 0:ch, :]).then_inc(s_out[b2], 16)
                if WARMUP:
                    sy.wait_ge(s_wg, 16)
                n_par = [sum(1 for t in range(NTILES) if t % 2 == b) for b in range(2)]
                for b in range(2):
                    sy.wait_ge(s_out[b], 16 * n_par[b])

    if compile_:
        nc.compile()
    return nc


def make_in_maps(indices: np.ndarray, emb_tables: np.ndarray):
    """Host-side sharding + index marshalling into dma_gather's layout.

    Returns (in_maps, inv_scale)."""
    idx = np.asarray(indices).astype(np.int64)  # [N_TOKENS, F]
    tab = np.ascontiguousarray(np.asarray(emb_tables), dtype=np.float32).reshape(
        F * VOCAB, D
    )
    if GATHER_DT == "i8":
        absmax = float(np.abs(tab).max())
        scale = Q_SCALE_NUM / absmax if absmax > 0 else 1.0
        qtab = np.clip(np.rint(tab * scale), -127, 127).astype(np.int8)
        inv_scale = np.float32(1.0 / scale)
    else:
        qtab = tab.astype(np.float16)
        inv_scale = np.float32(1.0)
    fused = (idx + (np.arange(F, dtype=np.int64) * VOCAB)[None, :]).astype(np.int16)

    in_maps = []
    for c in range(NCORES):
        sh = fused[c * TPC : (c + 1) * TPC]  # [TPC, F]
        cols = np.empty((16, IDX_COLS), dtype=np.int16)
        for t in range(NTILES):
            sz, ch = TSIZES[t], TSIZES[t] // 128
            # gather position i (in a feature block) holds token (i%128)*ch + i//128
            perm = (np.arange(sz) % 128) * ch + np.arange(sz) // 128
            tokp = sh[TOFFS[t] : TOFFS[t] + sz][perm]  # [sz, F]
            # queue q gathers features (q, q+4): block [q, 2*sz]
            blocks = np.concatenate(
                [tokp[:, 0:NQUEUES], tokp[:, NQUEUES:F]], axis=0
            ).T  # [q, 2*sz]
            # position j -> partition j%16, column j//16
            wrapped = blocks.reshape(NQUEUES, 2 * sz // 16, 16).transpose(2, 0, 1)
            c0 = TCOL_OFF[t]
            cols[:, c0 : c0 + NQUEUES * TCOLS[t]] = wrapped.reshape(16, -1)
        idx128 = np.ascontiguousarray(np.tile(cols, (8, 1)))
        in_maps.append({"tables": qtab, "idx": idx128})
    return in_maps, inv_scale


_NC = None


def kernel(indices: np.ndarray, emb_tables: np.ndarray) -> np.ndarray:
    global _NC
    from concourse.bass_utils import run_bass_kernel_spmd

    in_maps, inv_scale = make_in_maps(indices, emb_tables)
    if _NC is None:
        _NC = build_nc()
    res = run_bass_kernel_spmd(_NC, in_maps, core_ids=list(range(NCORES)))
    outs = [np.asarray(res.results[c]["out"]) for c in range(NCORES)]
    full = np.concatenate(outs, axis=0).astype(np.float32) * inv_scale
    return full.reshape(1, N_TOKENS, D)


# revision 42
# speedup vs baseline: 1.0382x; 1.0216x over previous
"""Embedding-lookup-sum kernel for TRN2 (8 NeuronCores, data-parallel).

out[n] = sum_f emb_tables[f, indices[n, f]]   (N=65536 tokens, F=8, D=256)

Strategy:
  - Shard tokens across 8 cores (8192 tokens/core); replicate the tables.
  - Fuse the 8 per-feature tables into one [8*1026, 256] DRAM table with the
    feature offset folded into the index on the host (idx + 1026*f).
  - Quantize the table to int8 with one global scale (absmax/126). The 8-way
    sum of int8 rows is exact in f16 (|sum| <= 1016 < 2048), so the host
    multiplies the f16 output by 1/scale afterwards. 256B gather rows halve
    both HBM traffic and SDMA per-descriptor time vs f16.
  - Per core: 8 tiles x 1024 tokens; per tile one 1024-idx dma_gather per
    (queue, feature) pair — queue q owns features q and q+4 (4 SWDGE queues).
    DVE folds the 8 feature blocks with a 4-add tree (int8 pair adds -> f16),
    sync DMAs the f16 sums out in 4KB/partition lines (token p*8+c at
    partition p, chunk c via a host-side index permutation).
  - A warmup gather absorbs the Q7 ucode cold start while the (split) idx
    load is in flight.

Measured bottleneck: dma_gather descriptor generation is effectively SERIAL
on the Q7s at ~2.8us per 1024-idx call (~2.7ns/desc: scalar idx unpack +
vector desc pushes), so 64 calls ~ 179us dominates; DMA drain (18.4ns/desc
per engine) and the DVE add tree (12.4us/tile) hide underneath. 2048-idx
calls exceed the per-queue descriptor ring and hang the device; bigger
dynamic_dma_scratch_size does not raise the effective ring.
"""

import sys

sys.path.insert(0, "/opt/trn_rl_repo")

import numpy as np

N_TOKENS = 65536
F = 8
VOCAB = 1026
D = 256
NCORES = 8
TPC = N_TOKENS // NCORES  # tokens per core = 8192
# Tile sizes. 1024-idx gathers are the largest the descriptor ring allows;
# uniform 1024-token tiles minimize call count (64), which matters because
# desc-gen is serial and each call carries ~0.35us fixed cost. (Tried
# [1024]*7+[256]*4 to shorten the dependency tail: the 24 extra calls cost
# more than the ~4us tail saving.)
TSIZES = [1024] * 8
assert sum(TSIZES) == TPC and all(sz % 128 == 0 for sz in TSIZES)
TOFFS = [sum(TSIZES[:j]) for j in range(len(TSIZES))]
NTILES = len(TSIZES)
NQUEUES = 4
GPQ = 2  # gather calls per queue per tile (features q and q+4)
# idx column offset of (tile j, queue q, call k): 16-wrapped positions
TCOLS = [2 * sz // 16 for sz in TSIZES]  # idx cols per (tile, queue)
TCOL_OFF = [sum(NQUEUES * c for c in TCOLS[:j]) for j in range(NTILES)]
IDX_COLS = sum(NQUEUES * c for c in TCOLS)  # total idx columns
NBUFG = 3  # gather buffer depth (tiles in flight)
SCRATCH = 32768  # dynamic DMA descriptor carveout bytes
WARMUP = True

GATHER_DT = "i8"  # "i8" (int8 + host scale) or "f16"
Q_SCALE_NUM = 126.0  # int8 quantization: scale = Q_SCALE_NUM / absmax


def build_nc(compile_: bool = True):
    import concourse.bacc as bacc
    import concourse.mybir as mybir
    from concourse.library_config import mlp
    from contextlib import ExitStack

    i8 = GATHER_DT == "i8"
    gdt = mybir.dt.int8 if i8 else mybir.dt.float16
    nbufg = NBUFG if i8 else 2

    nc = bacc.Bacc(
        "TRN2",
        debug=False,
        num_swdge_queues=NQUEUES,
        dynamic_dma_scratch_size=SCRATCH,
    )
    CHMAX = max(TSIZES) // 128

    tables = nc.dram_tensor("tables", [F * VOCAB, D], gdt, kind="ExternalInput")
    idx = nc.dram_tensor("idx", [128, IDX_COLS], mybir.dt.int16, kind="ExternalInput")
    out = nc.dram_tensor("out", [TPC, D], mybir.dt.float16, kind="ExternalOutput")

    with ExitStack() as ctx:
        idx_sb = ctx.enter_context(
            nc.sbuf_tensor("idx_sb", [128, IDX_COLS], mybir.dt.int16)
        )
        widx = ctx.enter_context(nc.sbuf_tensor("widx", [128, 8], mybir.dt.int16))
        gw = ctx.enter_context(nc.sbuf_tensor("gw", [128, 1, D], gdt))
        # g[b]: 8 feature blocks of ch chunks each (ch = tile_size/128);
        # queue q's two gathers fill blocks for features q and q+4.
        g = [
            ctx.enter_context(nc.sbuf_tensor(f"g{b}", [128, F * CHMAX, D], gdt))
            for b in range(nbufg)
        ]
        # s[b2]: f16 partial sums; final tile result lands in s[b2][:, 0:ch, :]
        s = [
            ctx.enter_context(
                nc.sbuf_tensor(f"s{b}", [128, 4 * CHMAX, D], mybir.dt.float16)
            )
            for b in range(2)
        ]
        s_w = ctx.enter_context(nc.semaphore("s_w"))
        s_wg = ctx.enter_context(nc.semaphore("s_wg"))
        s_idx0 = ctx.enter_context(nc.semaphore("s_idx0"))
        s_idxr = ctx.enter_context(nc.semaphore("s_idxr"))
        # Per-(buffer, queue) gather sems: a sem may only be updated from one
        # SWDGE queue, and count-based waits need all DMAs on a sem to be
        # "all issued so far" (completions can reorder).
        s_g = [
            [ctx.enter_context(nc.semaphore(f"s_g{b}_{q}")) for q in range(NQUEUES)]
            for b in range(nbufg)
        ]
        # q3's second gather gets its own sem so c3a/c3b can wait on the two
        # gathers independently (mixed +16s on one sem can't distinguish them)
        s_g3b = [ctx.enter_context(nc.semaphore(f"s_g3b{b}")) for b in range(nbufg)]
        s_ch = ctx.enter_context(nc.semaphore("s_ch"))  # +4/tile (c1, c2, c3a, c3b)
        s_sum = ctx.enter_context(nc.semaphore("s_sum"))  # +1/tile (c4)
        s_out = [ctx.enter_context(nc.semaphore(f"s_out{b}")) for b in range(2)]

        with nc.Block(no_gpsimd_drain=True) as block:

            @block.gpsimd
            def _(gp):
                gp.load_library(mlp)
                regs = {sz: gp.to_reg(sz) for sz in sorted(set(TSIZES))}
                if WARMUP:
                    wreg = gp.to_reg(16)
                    # Warmup: page in the dma_gather ucode while the idx DMA
                    # is in flight. Reads table row 0 (widx zeroed by DVE).
                    # 16 idxs: just enough to touch the gen code path.
                    gp.wait_ge(s_w, 1)
                    gp.dma_gather(
                        gw[:], tables[:], widx[:, 0:1], 16, wreg, D,
                        queue_num=0,
                    ).then_inc(s_wg, 16)
                for t in range(NTILES):
                    b = t % nbufg
                    sz = TSIZES[t]
                    ch = sz // 128
                    gcols = sz // 16
                    gp.wait_ge(s_idx0 if t == 0 else s_idxr, 16)
                    if t >= nbufg:
                        # g[b] free once tile t-nbufg's c3 consumed its last block
                        gp.wait_ge(s_ch, 4 * (t - nbufg + 1))
                    for q in range(NQUEUES):
                        for k in range(GPQ):
                            c0 = TCOL_OFF[t] + q * TCOLS[t] + k * gcols
                            ch0 = (q * 2 + k) * ch
                            sem = s_g3b[b] if (q == 3 and k == 1) else s_g[b][q]
                            gp.dma_gather(
                                g[b][:, ch0 : ch0 + ch, :],
                                tables[:],
                                idx_sb[:, c0 : c0 + gcols],
                                sz,
                                regs[sz],
                                D,
                                queue_num=q,
                                single_packet=False,
                            ).then_inc(sem, 16)

            @block.vector
            def _(v):
                if WARMUP:
                    v.memzero(widx[:]).then_inc(s_w, 1)
                for t in range(NTILES):
                    b = t % nbufg
                    b2 = t % 2
                    ch = TSIZES[t] // 128
                    # s[b2] free once tile t-2's out DMA read it
                    if t >= 2:
                        v.wait_ge(s_out[b2], 16 * (t // 2))
                        # ...and tile t-2's c4 (same buffer) fully retired
                        v.wait_ge(s_sum, t - 1)
                    # Serial accumulation: each queue's block folds in as soon
                    # as its gathers land, so only c3+c4 trail the last gather.
                    v.wait_ge(s_g[b][0], 16 * GPQ * (t // nbufg + 1))
                    v.wait_ge(s_g[b][1], 16 * GPQ * (t // nbufg + 1))
                    # c1: (f0+f1 | f4+f5) -> s[0:2ch]
                    v.tensor_add(
                        s[b2][:, 0 : 2 * ch, :],
                        g[b][:, 0 : 2 * ch, :],
                        g[b][:, 2 * ch : 4 * ch, :],
                    ).then_inc(s_ch, 1)
                    # c2: += (f2 | f6)
                    v.wait_ge(s_g[b][2], 16 * GPQ * (t // nbufg + 1))
                    v.wait_ge(s_ch, 4 * t + 1)
                    v.tensor_add(
                        s[b2][:, 0 : 2 * ch, :],
                        s[b2][:, 0 : 2 * ch, :],
                        g[b][:, 4 * ch : 6 * ch, :],
                    ).then_inc(s_ch, 1)
                    # c3a: += f3 (q3's first gather; disjoint from c3b's region,
                    # so the last-generated gather only gates the half-size c3b)
                    v.wait_ge(s_g[b][3], 16 * (t // nbufg + 1))
                    v.wait_ge(s_ch, 4 * t + 2)
                    v.tensor_add(
                        s[b2][:, 0:ch, :],
                        s[b2][:, 0:ch, :],
                        g[b][:, 6 * ch : 7 * ch, :],
                    ).then_inc(s_ch, 1)
                    # c3b: += f7 (q3's second, last-generated gather)
                    v.wait_ge(s_g3b[b], 16 * (t // nbufg + 1))
                    v.tensor_add(
                        s[b2][:, ch : 2 * ch, :],
                        s[b2][:, ch : 2 * ch, :],
                        g[b][:, 7 * ch : 8 * ch, :],
                    ).then_inc(s_ch, 1)
                    # c4: fold halves -> s[0:ch]
                    v.wait_ge(s_ch, 4 * t + 4)
                    v.tensor_add(
                        s[b2][:, 0:ch, :],
                        s[b2][:, 0:ch, :],
                        s[b2][:, ch : 2 * ch, :],
                    ).then_inc(s_sum, 1)

            @block.sync
            def _(sy):
                c_t0 = NQUEUES * TCOLS[0]
                sy.dma_start(idx_sb[:, 0:c_t0], idx[:, 0:c_t0]).then_inc(s_idx0, 16)
                sy.dma_start(idx_sb[:, c_t0:], idx[:, c_t0:]).then_inc(s_idxr, 16)
                for t in range(NTILES):
                    b2 = t % 2
                    ch = TSIZES[t] // 128
                    sy.wait_ge(s_sum, t + 1)
                    # token p*ch+c sits at partition p, chunk c -> 4KB HBM rows
                    dst = out[TOFFS[t] : TOFFS[t] + TSIZES[t], :].rearrange(
                        "(p c) d -> p c d", p=128
                    )
                    sy.dma_start(dst, s[b2][:, 0:ch, :]).then_inc(s_out[b2], 16)
                if WARMUP:
                    sy.wait_ge(s_wg, 16)
                n_par = [sum(1 for t in range(NTILES) if t % 2 == b) for b in range(2)]
                for b in range(2):
                    sy.wait_ge(s_out[b], 16 * n_par[b])

    if compile_:
        nc.compile()
    return nc


def make_in_maps(indices: np.ndarray, emb_tables: np.ndarray):
    """Host-side sharding + index marshalling into dma_gather's layout.

    Returns (in_maps, inv_scale)."""
    idx = np.asarray(indices).astype(np.int64)  # [N_TOKENS, F]
    tab = np.ascontiguousarray(np.asarray(emb_tables), dtype=np.float32).reshape(
        F * VOCAB, D
    )
    if GATHER_DT == "i8":
        absmax = float(np.abs(tab).max())
        scale = Q_SCALE_NUM / absmax if absmax > 0 else 1.0
        qtab = np.clip(np.rint(tab * scale), -127, 127).astype(np.int8)
        inv_scale = np.float32(1.0 / scale)
    else:
        qtab = tab.astype(np.float16)
        inv_scale = np.float32(1.0)
    fused = (idx + (np.arange(F, dtype=np.int64) * VOCAB)[None, :]).astype(np.int16)

    in_maps = []
    for c in range(NCORES):
        sh = fused[c * TPC : (c + 1) * TPC]  # [TPC, F]
        cols = np.empty((16, IDX_COLS), dtype=np.int16)
        for t in range(NTILES):
            sz, ch = TSIZES[t], TSIZES[t] // 128
            # gather position i (in a feature block) holds token (i%128)*ch + i//128
            perm = (np.arange(sz) % 128) * ch + np.arange(sz) // 128
            tokp = sh[TOFFS[t] : TOFFS[t] + sz][perm]  # [sz, F]
            # queue q gathers features (q, q+4): block [q, 2*sz]
            blocks = np.concatenate(
                [tokp[:, 0:NQUEUES], tokp[:, NQUEUES:F]], axis=0
            ).T  # [q, 2*sz]
            # position j -> partition j%16, column j//16
            wrapped = blocks.reshape(NQUEUES, 2 * sz // 16, 16).transpose(2, 0, 1)
            c0 = TCOL_OFF[t]
            cols[:, c0 : c0 + NQUEUES * TCOLS[t]] = wrapped.reshape(16, -1)
        idx128 = np.ascontiguousarray(np.tile(cols, (8, 1)))
        in_maps.append({"tables": qtab, "idx": idx128})
    return in_maps, inv_scale


_NC = None


def kernel(indices: np.ndarray, emb_tables: np.ndarray) -> np.ndarray:
    global _NC
    from concourse.bass_utils import run_bass_kernel_spmd

    in_maps, inv_scale = make_in_maps(indices, emb_tables)
    if _NC is None:
        _NC = build_nc()
    res = run_bass_kernel_spmd(_NC, in_maps, core_ids=list(range(NCORES)))
    outs = [np.asarray(res.results[c]["out"]) for c in range(NCORES)]
    full = np.concatenate(outs, axis=0).astype(np.float32) * inv_scale
    return full.reshape(1, N_TOKENS, D)


# revision 43
# speedup vs baseline: 1.0487x; 1.0101x over previous
"""Embedding-lookup-sum kernel for TRN2 (8 NeuronCores, data-parallel).

out[n] = sum_f emb_tables[f, indices[n, f]]   (N=65536 tokens, F=8, D=256)

Strategy:
  - Shard tokens across 8 cores (8192 tokens/core); replicate the tables.
  - Fuse the 8 per-feature tables into one [8*1026, 256] DRAM table with the
    feature offset folded into the index on the host (idx + 1026*f).
  - Quantize the table to int8 with one global scale (absmax/126). The 8-way
    sum of int8 rows is exact in f16 (|sum| <= 1016 < 2048), so the host
    multiplies the f16 output by 1/scale afterwards. 256B gather rows halve
    both HBM traffic and SDMA per-descriptor time vs f16.
  - Per core: 8 tiles x 1024 tokens; per tile one 1024-idx dma_gather per
    (queue, feature) pair — queue q owns features q and q+4 (4 SWDGE queues).
    DVE folds the 8 feature blocks with a serial chain (c1=q0+q1, c2+=q2,
    c3a+=f3, c3b+=f7, c4 fold); c3a/c3b write disjoint halves so the
    last-generated gather only gates the half-size c3b, shortening the end
    tail. Sync DMAs the f16 sums out in 4KB/partition lines (token p*8+c at
    partition p, chunk c via a host-side index permutation).
  - A warmup gather absorbs the Q7 ucode cold start while the (split) idx
    load is in flight.

Measured bottleneck: dma_gather descriptor generation is effectively SERIAL
on the Q7s at ~2.8us per 1024-idx call (~2.7ns/desc: scalar idx unpack +
vector desc pushes), so 64 calls ~ 179us dominates; DMA drain (18.4ns/desc
per engine) and the DVE add tree (12.4us/tile) hide underneath. 2048-idx
calls exceed the per-queue descriptor ring and hang the device; bigger
dynamic_dma_scratch_size does not raise the effective ring.
"""

import sys

sys.path.insert(0, "/opt/trn_rl_repo")

import numpy as np

N_TOKENS = 65536
F = 8
VOCAB = 1026
D = 256
NCORES = 8
TPC = N_TOKENS // NCORES  # tokens per core = 8192
# Tile sizes. 1024-idx gathers are the largest the descriptor ring allows;
# uniform 1024-token tiles minimize call count (64), which matters because
# desc-gen is serial and each call carries ~0.35us fixed cost. (Tried
# [1024]*7+[256]*4 to shorten the dependency tail: the 24 extra calls cost
# more than the ~4us tail saving.)
TSIZES = [1024] * 8
assert sum(TSIZES) == TPC and all(sz % 128 == 0 for sz in TSIZES)
TOFFS = [sum(TSIZES[:j]) for j in range(len(TSIZES))]
NTILES = len(TSIZES)
NQUEUES = 4
GPQ = 2  # gather calls per queue per tile (features q and q+4)
# idx column offset of (tile j, queue q, call k): 16-wrapped positions
TCOLS = [2 * sz // 16 for sz in TSIZES]  # idx cols per (tile, queue)
TCOL_OFF = [sum(NQUEUES * c for c in TCOLS[:j]) for j in range(NTILES)]
IDX_COLS = sum(NQUEUES * c for c in TCOLS)  # total idx columns
NBUFG = 3  # gather buffer depth (tiles in flight)
SCRATCH = 32768  # dynamic DMA descriptor carveout bytes
WARMUP = True

GATHER_DT = "i8"  # "i8" (int8 + host scale) or "f16"
Q_SCALE_NUM = 126.0  # int8 quantization: scale = Q_SCALE_NUM / absmax


def build_nc(compile_: bool = True):
    import concourse.bacc as bacc
    import concourse.mybir as mybir
    from concourse.library_config import mlp
    from contextlib import ExitStack

    i8 = GATHER_DT == "i8"
    gdt = mybir.dt.int8 if i8 else mybir.dt.float16
    nbufg = NBUFG if i8 else 2

    nc = bacc.Bacc(
        "TRN2",
        debug=False,
        num_swdge_queues=NQUEUES,
        dynamic_dma_scratch_size=SCRATCH,
    )
    CHMAX = max(TSIZES) // 128

    tables = nc.dram_tensor("tables", [F * VOCAB, D], gdt, kind="ExternalInput")
    idx = nc.dram_tensor("idx", [128, IDX_COLS], mybir.dt.int16, kind="ExternalInput")
    out = nc.dram_tensor("out", [TPC, D], mybir.dt.float16, kind="ExternalOutput")

    with ExitStack() as ctx:
        idx_sb = ctx.enter_context(
            nc.sbuf_tensor("idx_sb", [128, IDX_COLS], mybir.dt.int16)
        )
        widx = ctx.enter_context(nc.sbuf_tensor("widx", [128, 8], mybir.dt.int16))
        gw = ctx.enter_context(nc.sbuf_tensor("gw", [128, 1, D], gdt))
        # g[b]: 8 feature blocks of ch chunks each (ch = tile_size/128);
        # queue q's two gathers fill blocks for features q and q+4.
        g = [
            ctx.enter_context(nc.sbuf_tensor(f"g{b}", [128, F * CHMAX, D], gdt))
            for b in range(nbufg)
        ]
        # s[b2]: f16 partial sums; final tile result lands in s[b2][:, 0:ch, :]
        s = [
            ctx.enter_context(
                nc.sbuf_tensor(f"s{b}", [128, 4 * CHMAX, D], mybir.dt.float16)
            )
            for b in range(2)
        ]
        s_w = ctx.enter_context(nc.semaphore("s_w"))
        s_wg = ctx.enter_context(nc.semaphore("s_wg"))
        s_idx0 = ctx.enter_context(nc.semaphore("s_idx0"))
        s_idxr = ctx.enter_context(nc.semaphore("s_idxr"))
        # Per-(buffer, queue) gather sems: a sem may only be updated from one
        # SWDGE queue, and count-based waits need all DMAs on a sem to be
        # "all issued so far" (completions can reorder).
        s_g = [
            [ctx.enter_context(nc.semaphore(f"s_g{b}_{q}")) for q in range(NQUEUES)]
            for b in range(nbufg)
        ]
        # q3's second gather gets its own sem so c3a/c3b can wait on the two
        # gathers independently (mixed +16s on one sem can't distinguish them)
        s_g3b = [ctx.enter_context(nc.semaphore(f"s_g3b{b}")) for b in range(nbufg)]
        s_ch = ctx.enter_context(nc.semaphore("s_ch"))  # +4/tile (c1, c2, c3a, c3b)
        s_sum = ctx.enter_context(nc.semaphore("s_sum"))  # +1/tile (c4)
        s_out = [ctx.enter_context(nc.semaphore(f"s_out{b}")) for b in range(2)]

        with nc.Block(no_gpsimd_drain=True) as block:

            @block.gpsimd
            def _(gp):
                gp.load_library(mlp)
                regs = {sz: gp.to_reg(sz) for sz in sorted(set(TSIZES))}
                if WARMUP:
                    wreg = gp.to_reg(16)
                    # Warmup: page in the dma_gather ucode while the idx DMA
                    # is in flight. Reads table row 0 (widx zeroed by DVE).
                    # 16 idxs: just enough to touch the gen code path.
                    gp.wait_ge(s_w, 1)
                    gp.dma_gather(
                        gw[:], tables[:], widx[:, 0:1], 16, wreg, D,
                        queue_num=0,
                    ).then_inc(s_wg, 16)
                for t in range(NTILES):
                    b = t % nbufg
                    sz = TSIZES[t]
                    ch = sz // 128
                    gcols = sz // 16
                    gp.wait_ge(s_idx0 if t == 0 else s_idxr, 16)
                    if t >= nbufg:
                        # g[b] free once tile t-nbufg's c3 consumed its last block
                        gp.wait_ge(s_ch, 4 * (t - nbufg + 1))
                    for q in range(NQUEUES):
                        for k in range(GPQ):
                            c0 = TCOL_OFF[t] + q * TCOLS[t] + k * gcols
                            ch0 = (q * 2 + k) * ch
                            sem = s_g3b[b] if (q == 3 and k == 1) else s_g[b][q]
                            gp.dma_gather(
                                g[b][:, ch0 : ch0 + ch, :],
                                tables[:],
                                idx_sb[:, c0 : c0 + gcols],
                                sz,
                                regs[sz],
                                D,
                                queue_num=q,
                                single_packet=False,
                            ).then_inc(sem, 16)

            @block.vector
            def _(v):
                if WARMUP:
                    v.memzero(widx[:]).then_inc(s_w, 1)
                for t in range(NTILES):
                    b = t % nbufg
                    b2 = t % 2
                    ch = TSIZES[t] // 128
                    # s[b2] free once tile t-2's out DMA read it
                    if t >= 2:
                        v.wait_ge(s_out[b2], 16 * (t // 2))
                        # ...and tile t-2's c4 (same buffer) fully retired
                        v.wait_ge(s_sum, t - 1)
                    # Serial accumulation: each queue's block folds in as soon
                    # as its gathers land, so only c3+c4 trail the last gather.
                    v.wait_ge(s_g[b][0], 16 * GPQ * (t // nbufg + 1))
                    v.wait_ge(s_g[b][1], 16 * GPQ * (t // nbufg + 1))
                    # c1: (f0+f1 | f4+f5) -> s[0:2ch]
                    v.tensor_add(
                        s[b2][:, 0 : 2 * ch, :],
                        g[b][:, 0 : 2 * ch, :],
                        g[b][:, 2 * ch : 4 * ch, :],
                    ).then_inc(s_ch, 1)
                    # c2: += (f2 | f6)
                    v.wait_ge(s_g[b][2], 16 * GPQ * (t // nbufg + 1))
                    v.wait_ge(s_ch, 4 * t + 1)
                    v.tensor_add(
                        s[b2][:, 0 : 2 * ch, :],
                        s[b2][:, 0 : 2 * ch, :],
                        g[b][:, 4 * ch : 6 * ch, :],
                    ).then_inc(s_ch, 1)
                    # c3a: += f3 (q3's first gather; disjoint from c3b's region,
                    # so the last-generated gather only gates the half-size c3b)
                    v.wait_ge(s_g[b][3], 16 * (t // nbufg + 1))
                    v.wait_ge(s_ch, 4 * t + 2)
                    v.tensor_add(
                        s[b2][:, 0:ch, :],
                        s[b2][:, 0:ch, :],
                        g[b][:, 6 * ch : 7 * ch, :],
                    ).then_inc(s_ch, 1)
                    # c3b: += f7 (q3's second, last-generated gather)
                    v.wait_ge(s_g3b[b], 16 * (t // nbufg + 1))
                    v.tensor_add(
                        s[b2][:, ch : 2 * ch, :],
                        s[b2][:, ch : 2 * ch, :],
                        g[b][:, 7 * ch : 8 * ch, :],
                    ).then_inc(s_ch, 1)
                    # c4: fold halves -> s[0:ch]
                    v.wait_ge(s_ch, 4 * t + 4)
                    v.tensor_add(
                        s[b2][:, 0:ch, :],
                        s[b2][:, 0:ch, :],
                        s[b2][:, ch : 2 * ch, :],
                    ).then_inc(s_sum, 1)

            @block.sync
            def _(sy):
                c_t0 = NQUEUES * TCOLS[0]
                sy.dma_start(idx_sb[:, 0:c_t0], idx[:, 0:c_t0]).then_inc(s_idx0, 16)
                sy.dma_start(idx_sb[:, c_t0:], idx[:, c_t0:]).then_inc(s_idxr, 16)
                for t in range(NTILES):
                    b2 = t % 2
                    ch = TSIZES[t] // 128
                    sy.wait_ge(s_sum, t + 1)
                    # token p*ch+c sits at partition p, chunk c -> 4KB HBM rows
                    dst = out[TOFFS[t] : TOFFS[t] + TSIZES[t], :].rearrange(
                        "(p c) d -> p c d", p=128
                    )
                    sy.dma_start(dst, s[b2][:, 0:ch, :]).then_inc(s_out[b2], 16)
                if WARMUP:
                    sy.wait_ge(s_wg, 16)
                n_par = [sum(1 for t in range(NTILES) if t % 2 == b) for b in range(2)]
                for b in range(2):
                    sy.wait_ge(s_out[b], 16 * n_par[b])

    if compile_:
        nc.compile()
    return nc


def make_in_maps(indices: np.ndarray, emb_tables: np.ndarray):
    """Host-side sharding + index marshalling into dma_gather's layout.

    Returns (in_maps, inv_scale)."""
    idx = np.asarray(indices).astype(np.int64)  # [N_TOKENS, F]
    tab = np.ascontiguousarray(np.asarray(emb_tables), dtype=np.float32).reshape(
        F * VOCAB, D
    )
    if GATHER_DT == "i8":
        absmax = float(np.abs(tab).max())
        scale = Q_SCALE_NUM / absmax if absmax > 0 else 1.0
        qtab = np.clip(np.rint(tab * scale), -127, 127).astype(np.int8)
        inv_scale = np.float32(1.0 / scale)
    else:
        qtab = tab.astype(np.float16)
        inv_scale = np.float32(1.0)
    fused = (idx + (np.arange(F, dtype=np.int64) * VOCAB)[None, :]).astype(np.int16)

    in_maps = []
    for c in range(NCORES):
        sh = fused[c * TPC : (c + 1) * TPC]  # [TPC, F]
        cols = np.empty((16, IDX_COLS), dtype=np.int16)
        for t in range(NTILES):
            sz, ch = TSIZES[t], TSIZES[t] // 128
            # gather position i (in a feature block) holds token (i%128)*ch + i//128
            perm = (np.arange(sz) % 128) * ch + np.arange(sz) // 128
            tokp = sh[TOFFS[t] : TOFFS[t] + sz][perm]  # [sz, F]
            # queue q gathers features (q, q+4): block [q, 2*sz]
            blocks = np.concatenate(
                [tokp[:, 0:NQUEUES], tokp[:, NQUEUES:F]], axis=0
            ).T  # [q, 2*sz]
            # position j -> partition j%16, column j//16
            wrapped = blocks.reshape(NQUEUES, 2 * sz // 16, 16).transpose(2, 0, 1)
            c0 = TCOL_OFF[t]
            cols[:, c0 : c0 + NQUEUES * TCOLS[t]] = wrapped.reshape(16, -1)
        idx128 = np.ascontiguousarray(np.tile(cols, (8, 1)))
        in_maps.append({"tables": qtab, "idx": idx128})
    return in_maps, inv_scale


_NC = None


def kernel(indices: np.ndarray, emb_tables: np.ndarray) -> np.ndarray:
    global _NC
    from concourse.bass_utils import run_bass_kernel_spmd

    in_maps, inv_scale = make_in_maps(indices, emb_tables)
    if _NC is None:
        _NC = build_nc()
    res = run_bass_kernel_spmd(_NC, in_maps, core_ids=list(range(NCORES)))
    outs = [np.asarray(res.results[c]["out"]) for c in range(NCORES)]
    full = np.concatenate(outs, axis=0).astype(np.float32) * inv_scale
    return full.reshape(1, N_TOKENS, D)


# revision 44
# speedup vs baseline: 1.0692x; 1.0195x over previous
"""Embedding-lookup-sum kernel for TRN2 (8 NeuronCores, data-parallel).

out[n] = sum_f emb_tables[f, indices[n, f]]   (N=65536 tokens, F=8, D=256)

Strategy:
  - Shard tokens across 8 cores (8192 tokens/core); replicate the tables.
  - Fuse the 8 per-feature tables into one [8*1026, 256] DRAM table with the
    feature offset folded into the index on the host (idx + 1026*f).
  - Quantize the table to int8 with one global scale (absmax/126). The 8-way
    sum of int8 rows is exact in f16 (|sum| <= 1016 < 2048), so the host
    multiplies the f16 output by 1/scale afterwards. 256B gather rows halve
    both HBM traffic and SDMA per-descriptor time vs f16.
  - Per core: 8 tiles x 1024 tokens; per tile one 1024-idx dma_gather per
    (queue, feature) pair — queue q owns features q and q+4 (4 SWDGE queues).
    DVE folds the 8 feature blocks with a serial chain (c1=q0+q1, c2+=q2,
    c3a+=f3, c3b+=f7, c4 fold); c3a/c3b write disjoint halves so the
    last-generated gather only gates the half-size c3b, shortening the end
    tail. Sync DMAs the f16 sums out in 4KB/partition lines (token p*8+c at
    partition p, chunk c via a host-side index permutation).
  - A warmup gather absorbs the Q7 ucode cold start while the (split) idx
    load is in flight.

Measured bottleneck: dma_gather descriptor generation is effectively SERIAL
on the Q7s at ~2.8us per 1024-idx call (~2.7ns/desc: scalar idx unpack +
vector desc pushes), so 64 calls ~ 179us dominates; DMA drain (18.4ns/desc
per engine) and the DVE add tree (12.4us/tile) hide underneath. 2048-idx
calls exceed the per-queue descriptor ring and hang the device; bigger
dynamic_dma_scratch_size does not raise the effective ring.
"""

import sys

sys.path.insert(0, "/opt/trn_rl_repo")

import numpy as np

N_TOKENS = 65536
F = 8
VOCAB = 1026
D = 256
NCORES = 8
TPC = N_TOKENS // NCORES  # tokens per core = 8192
# Tile sizes. 1024-idx gathers are the largest the descriptor ring allows;
# uniform 1024-token tiles minimize call count (64), which matters because
# desc-gen is serial and each call carries ~0.35us fixed cost. (Tried
# [1024]*7+[256]*4 to shorten the dependency tail: the 24 extra calls cost
# more than the ~4us tail saving.)
TSIZES = [1024] * 8
assert sum(TSIZES) == TPC and all(sz % 128 == 0 for sz in TSIZES)
TOFFS = [sum(TSIZES[:j]) for j in range(len(TSIZES))]
NTILES = len(TSIZES)
NQUEUES = 4
GPQ = 2  # gather calls per queue per tile (features q and q+4)
# idx column offset of (tile j, queue q, call k): 16-wrapped positions
TCOLS = [2 * sz // 16 for sz in TSIZES]  # idx cols per (tile, queue)
TCOL_OFF = [sum(NQUEUES * c for c in TCOLS[:j]) for j in range(NTILES)]
IDX_COLS = sum(NQUEUES * c for c in TCOLS)  # total idx columns
NBUFG = 3  # gather buffer depth (tiles in flight)
SCRATCH = 32768  # dynamic DMA descriptor carveout bytes
WARMUP = False

GATHER_DT = "i8"  # "i8" (int8 + host scale) or "f16"
Q_SCALE_NUM = 126.0  # int8 quantization: scale = Q_SCALE_NUM / absmax


def build_nc(compile_: bool = True):
    import concourse.bacc as bacc
    import concourse.mybir as mybir
    from concourse.library_config import mlp
    from contextlib import ExitStack

    i8 = GATHER_DT == "i8"
    gdt = mybir.dt.int8 if i8 else mybir.dt.float16
    nbufg = NBUFG if i8 else 2

    nc = bacc.Bacc(
        "TRN2",
        debug=False,
        num_swdge_queues=NQUEUES,
        dynamic_dma_scratch_size=SCRATCH,
    )
    CHMAX = max(TSIZES) // 128

    tables = nc.dram_tensor("tables", [F * VOCAB, D], gdt, kind="ExternalInput")
    idx = nc.dram_tensor("idx", [128, IDX_COLS], mybir.dt.int16, kind="ExternalInput")
    out = nc.dram_tensor("out", [TPC, D], mybir.dt.float16, kind="ExternalOutput")

    with ExitStack() as ctx:
        idx_sb = ctx.enter_context(
            nc.sbuf_tensor("idx_sb", [128, IDX_COLS], mybir.dt.int16)
        )
        widx = ctx.enter_context(nc.sbuf_tensor("widx", [128, 8], mybir.dt.int16))
        gw = ctx.enter_context(nc.sbuf_tensor("gw", [128, 1, D], gdt))
        # g[b]: 8 feature blocks of ch chunks each (ch = tile_size/128);
        # queue q's two gathers fill blocks for features q and q+4.
        g = [
            ctx.enter_context(nc.sbuf_tensor(f"g{b}", [128, F * CHMAX, D], gdt))
            for b in range(nbufg)
        ]
        # s[b2]: f16 partial sums; final tile result lands in s[b2][:, 0:ch, :]
        s = [
            ctx.enter_context(
                nc.sbuf_tensor(f"s{b}", [128, 4 * CHMAX, D], mybir.dt.float16)
            )
            for b in range(2)
        ]
        s_w = ctx.enter_context(nc.semaphore("s_w"))
        s_wg = ctx.enter_context(nc.semaphore("s_wg"))
        s_idx0 = ctx.enter_context(nc.semaphore("s_idx0"))
        s_idxr = ctx.enter_context(nc.semaphore("s_idxr"))
        # Per-(buffer, queue) gather sems: a sem may only be updated from one
        # SWDGE queue, and count-based waits need all DMAs on a sem to be
        # "all issued so far" (completions can reorder).
        s_g = [
            [ctx.enter_context(nc.semaphore(f"s_g{b}_{q}")) for q in range(NQUEUES)]
            for b in range(nbufg)
        ]
        # q3's second gather gets its own sem so c3a/c3b can wait on the two
        # gathers independently (mixed +16s on one sem can't distinguish them)
        s_g3b = [ctx.enter_context(nc.semaphore(f"s_g3b{b}")) for b in range(nbufg)]
        s_ch = ctx.enter_context(nc.semaphore("s_ch"))  # +4/tile (c1, c2, c3a, c3b)
        s_sum = ctx.enter_context(nc.semaphore("s_sum"))  # +1/tile (c4)
        s_out = [ctx.enter_context(nc.semaphore(f"s_out{b}")) for b in range(2)]

        with nc.Block(no_gpsimd_drain=True) as block:

            @block.gpsimd
            def _(gp):
                gp.load_library(mlp)
                regs = {sz: gp.to_reg(sz) for sz in sorted(set(TSIZES))}
                if WARMUP:
                    wreg = gp.to_reg(16)
                    # Warmup: page in the dma_gather ucode while the idx DMA
                    # is in flight. Reads table row 0 (widx zeroed by DVE).
                    # 16 idxs: just enough to touch the gen code path.
                    gp.wait_ge(s_w, 1)
                    gp.dma_gather(
                        gw[:], tables[:], widx[:, 0:1], 16, wreg, D,
                        queue_num=0,
                    ).then_inc(s_wg, 16)
                for t in range(NTILES):
                    b = t % nbufg
                    sz = TSIZES[t]
                    ch = sz // 128
                    gcols = sz // 16
                    gp.wait_ge(s_idx0 if t == 0 else s_idxr, 16)
                    if t >= nbufg:
                        # g[b] free once tile t-nbufg's c3 consumed its last block
                        gp.wait_ge(s_ch, 4 * (t - nbufg + 1))
                    for q in range(NQUEUES):
                        for k in range(GPQ):
                            c0 = TCOL_OFF[t] + q * TCOLS[t] + k * gcols
                            ch0 = (q * 2 + k) * ch
                            sem = s_g3b[b] if (q == 3 and k == 1) else s_g[b][q]
                            gp.dma_gather(
                                g[b][:, ch0 : ch0 + ch, :],
                                tables[:],
                                idx_sb[:, c0 : c0 + gcols],
                                sz,
                                regs[sz],
                                D,
                                queue_num=q,
                                single_packet=False,
                            ).then_inc(sem, 16)

            @block.vector
            def _(v):
                if WARMUP:
                    v.memzero(widx[:]).then_inc(s_w, 1)
                for t in range(NTILES):
                    b = t % nbufg
                    b2 = t % 2
                    ch = TSIZES[t] // 128
                    # s[b2] free once tile t-2's out DMA read it
                    if t >= 2:
                        v.wait_ge(s_out[b2], 16 * (t // 2))
                        # ...and tile t-2's c4 (same buffer) fully retired
                        v.wait_ge(s_sum, t - 1)
                    # Serial accumulation: each queue's block folds in as soon
                    # as its gathers land, so only c3+c4 trail the last gather.
                    v.wait_ge(s_g[b][0], 16 * GPQ * (t // nbufg + 1))
                    v.wait_ge(s_g[b][1], 16 * GPQ * (t // nbufg + 1))
                    # c1: (f0+f1 | f4+f5) -> s[0:2ch]
                    v.tensor_add(
                        s[b2][:, 0 : 2 * ch, :],
                        g[b][:, 0 : 2 * ch, :],
                        g[b][:, 2 * ch : 4 * ch, :],
                    ).then_inc(s_ch, 1)
                    # c2: += (f2 | f6)
                    v.wait_ge(s_g[b][2], 16 * GPQ * (t // nbufg + 1))
                    v.wait_ge(s_ch, 4 * t + 1)
                    v.tensor_add(
                        s[b2][:, 0 : 2 * ch, :],
                        s[b2][:, 0 : 2 * ch, :],
                        g[b][:, 4 * ch : 6 * ch, :],
                    ).then_inc(s_ch, 1)
                    # c3a: += f3 (q3's first gather; disjoint from c3b's region,
                    # so the last-generated gather only gates the half-size c3b)
                    v.wait_ge(s_g[b][3], 16 * (t // nbufg + 1))
                    v.wait_ge(s_ch, 4 * t + 2)
                    v.tensor_add(
                        s[b2][:, 0:ch, :],
                        s[b2][:, 0:ch, :],
                        g[b][:, 6 * ch : 7 * ch, :],
                    ).then_inc(s_ch, 1)
                    # c3b: += f7 (q3's second, last-generated gather)
                    v.wait_ge(s_g3b[b], 16 * (t // nbufg + 1))
                    v.tensor_add(
                        s[b2][:, ch : 2 * ch, :],
                        s[b2][:, ch : 2 * ch, :],
                        g[b][:, 7 * ch : 8 * ch, :],
                    ).then_inc(s_ch, 1)
                    # c4: fold halves -> s[0:ch]
                    v.wait_ge(s_ch, 4 * t + 4)
                    v.tensor_add(
                        s[b2][:, 0:ch, :],
                        s[b2][:, 0:ch, :],
                        s[b2][:, ch : 2 * ch, :],
                    ).then_inc(s_sum, 1)

            @block.sync
            def _(sy):
                c_t0 = NQUEUES * TCOLS[0]
                sy.dma_start(idx_sb[:, 0:c_t0], idx[:, 0:c_t0]).then_inc(s_idx0, 16)
                sy.dma_start(idx_sb[:, c_t0:], idx[:, c_t0:]).then_inc(s_idxr, 16)
                for t in range(NTILES):
                    b2 = t % 2
                    ch = TSIZES[t] // 128
                    sy.wait_ge(s_sum, t + 1)
                    # token p*ch+c sits at partition p, chunk c -> 4KB HBM rows
                    dst = out[TOFFS[t] : TOFFS[t] + TSIZES[t], :].rearrange(
                        "(p c) d -> p c d", p=128
                    )
                    sy.dma_start(dst, s[b2][:, 0:ch, :]).then_inc(s_out[b2], 16)
                if WARMUP:
                    sy.wait_ge(s_wg, 16)
                n_par = [sum(1 for t in range(NTILES) if t % 2 == b) for b in range(2)]
                for b in range(2):
                    sy.wait_ge(s_out[b], 16 * n_par[b])

    if compile_:
        nc.compile()
    return nc


def make_in_maps(indices: np.ndarray, emb_tables: np.ndarray):
    """Host-side sharding + index marshalling into dma_gather's layout.

    Returns (in_maps, inv_scale)."""
    idx = np.asarray(indices).astype(np.int64)  # [N_TOKENS, F]
    tab = np.ascontiguousarray(np.asarray(emb_tables), dtype=np.float32).reshape(
        F * VOCAB, D
    )
    if GATHER_DT == "i8":
        absmax = float(np.abs(tab).max())
        scale = Q_SCALE_NUM / absmax if absmax > 0 else 1.0
        qtab = np.clip(np.rint(tab * scale), -127, 127).astype(np.int8)
        inv_scale = np.float32(1.0 / scale)
    else:
        qtab = tab.astype(np.float16)
        inv_scale = np.float32(1.0)
    fused = (idx + (np.arange(F, dtype=np.int64) * VOCAB)[None, :]).astype(np.int16)

    in_maps = []
    for c in range(NCORES):
        sh = fused[c * TPC : (c + 1) * TPC]  # [TPC, F]
        cols = np.empty((16, IDX_COLS), dtype=np.int16)
        for t in range(NTILES):
            sz, ch = TSIZES[t], TSIZES[t] // 128
            # gather position i (in a feature block) holds token (i%128)*ch + i//128
            perm = (np.arange(sz) % 128) * ch + np.arange(sz) // 128
            tokp = sh[TOFFS[t] : TOFFS[t] + sz][perm]  # [sz, F]
            # queue q gathers features (q, q+4): block [q, 2*sz]
            blocks = np.concatenate(
                [tokp[:, 0:NQUEUES], tokp[:, NQUEUES:F]], axis=0
            ).T  # [q, 2*sz]
            # position j -> partition j%16, column j//16
            wrapped = blocks.reshape(NQUEUES, 2 * sz // 16, 16).transpose(2, 0, 1)
            c0 = TCOL_OFF[t]
            cols[:, c0 : c0 + NQUEUES * TCOLS[t]] = wrapped.reshape(16, -1)
        idx128 = np.ascontiguousarray(np.tile(cols, (8, 1)))
        in_maps.append({"tables": qtab, "idx": idx128})
    return in_maps, inv_scale


_NC = None


def kernel(indices: np.ndarray, emb_tables: np.ndarray) -> np.ndarray:
    global _NC
    from concourse.bass_utils import run_bass_kernel_spmd

    in_maps, inv_scale = make_in_maps(indices, emb_tables)
    if _NC is None:
        _NC = build_nc()
    res = run_bass_kernel_spmd(_NC, in_maps, core_ids=list(range(NCORES)))
    outs = [np.asarray(res.results[c]["out"]) for c in range(NCORES)]
    full = np.concatenate(outs, axis=0).astype(np.float32) * inv_scale
    return full.reshape(1, N_TOKENS, D)
